# revision 3
# baseline (speedup 1.0000x reference)
"""MoE layer (B=4,S=2048,H=2048,E=4,K=2,FF=8192) on 8 TRN2 NeuronCores.

Strategy: token-parallel. Each core takes 1024 tokens and all 4 experts'
weights, computes the router + dense (masked) expert FFNs locally, and
writes a disjoint slice of the output. No collectives; host just
concatenates slices and reduces the tiny aux-loss partials.

Numerics: router in fp32 (exact top-k), FFN matmuls in float32r
(~1e-4 rel err, full PE rate).
"""

import json
import os

import numpy as np

import concourse.bass as bass
import concourse.mybir as mybir
import concourse.tile as tile_mod
from concourse.bass_utils import run_bass_kernel_spmd
from concourse.tile import TileContext
from concourse.vector_clock import ScopedClock

# ─────────────────────────────────────────────────────────────────────────────
# Workaround for this walrus build's 1-sync-wait-per-instruction limit.
_MAX_WAITS = 1


def _patched_drain_and_barrier(self, tick_clock, wait_clock):
    probe = self.nc.sync.nop(nofuse=True)
    wait_clock.add_sem_waits(probe.ins, ScopedClock({None: tick_clock.global_clock}))
    si = probe.ins.sync_info
    waits = list(si.on_wait) if si is not None else []
    if len(waits) > 1:
        probe.ins.sync_info = mybir.SyncInfo(
            on_wait=[waits[0]], on_update=list(si.on_update)
        )
        for w in waits[1:]:
            n = self.nc.sync.nop(nofuse=True)
            n.ins.sync_info = mybir.SyncInfo(on_wait=[w], on_update=[])
    self.nc.sync.drain()
    self.nc.all_engine_barrier()
    assert self.sems is not None
    popped = self.nc._tile_sem_poison_stack.pop()
    assert popped is self._sem_poison
    self.nc.clear_and_free_semaphores(list(self.sems.allocated().values()))


def _split_waits_in_bir_json(bir_json: bytes) -> bytes:
    m = json.loads(bir_json)
    changed = False
    for f in m.get("functions", []):
        for b in f.get("blocks", []):
            insts = b.get("instructions", [])
            out = []
            for inst in insts:
                si = inst.get("sync_info") or {}
                waits = si.get("on_wait") or []
                if len(waits) > _MAX_WAITS:
                    changed = True
                    extra, keep = waits[:-_MAX_WAITS], waits[-_MAX_WAITS:]
                    for j, w in enumerate(extra):
                        out.append(
                            {
                                "debug": inst.get("debug"),
                                "engine": inst["engine"],
                                "ins": [],
                                "name": f"{inst['name']}-wsplit{j}",
                                "opcode": "NoOp",
                                "outs": [],
                                "sync_info": {"on_update": [], "on_wait": [w]},
                            }
                        )
                    si = dict(si)
                    si["on_wait"] = keep
                    inst = dict(inst)
                    inst["sync_info"] = si
                out.append(inst)
            b["instructions"] = out
    if not changed:
        return bir_json
    return json.dumps(m).encode()


def _install_fixes():
    tile_mod.TileContext._drain_and_barrier = _patched_drain_and_barrier
    import concourse.bass_utils as bu
    import concourse.bass2jax as b2j

    if getattr(bu, "_wsplit_installed", False):
        return
    orig = bu.compile_bir_kernel

    def wrapped(bir_json, tmpdir, neff_name="file.neff"):
        return orig(_split_waits_in_bir_json(bytes(bir_json)), tmpdir, neff_name)

    bu.compile_bir_kernel = wrapped
    bu._wsplit_installed = True
    if getattr(b2j, "compile_bir_kernel", None) is orig:
        b2j.compile_bir_kernel = wrapped


_install_fixes()

# ─────────────────────────────────────────────────────────────────────────────

F32 = mybir.dt.float32
F32R = mybir.dt.float32r
A = mybir.AluOpType
AX = mybir.AxisListType
AF = mybir.ActivationFunctionType

B, S, H = 4, 2048, 2048
E, K = 4, 2
FF = 4 * H
NCORES = 8
N_TOK = B * S
T = N_TOK // NCORES  # 1024 tokens per core
T8 = T // 128  # 8
NH = H // 128  # 16
FB = 512  # FF block
NB = FF // FB  # 8
FM = FB // 128  # 8
HN = H // 512  # 4
TN = T // 512  # 2


def build():
    nc = bass.Bass()
    hs = nc.dram_tensor("hs", [T, H], F32, kind="ExternalInput")
    wg = nc.dram_tensor("wg", [H, E], F32, kind="ExternalInput")
    bg = nc.dram_tensor("bg", [1, E], F32, kind="ExternalInput")
    w1 = nc.dram_tensor("w1", [E, H, FF], F32R, kind="ExternalInput")
    b1 = nc.dram_tensor("b1", [E, FF], F32, kind="ExternalInput")
    w2 = nc.dram_tensor("w2", [E, FF, H], F32R, kind="ExternalInput")
    b2 = nc.dram_tensor("b2", [E, H], F32R, kind="ExternalInput")
    consts = nc.dram_tensor("consts", [128, 130], F32, kind="ExternalInput")
    ones_row = nc.dram_tensor("ones_row", [1, 128], F32, kind="ExternalInput")
    sel_mat = nc.dram_tensor("sel_mat", [4, 512], F32, kind="ExternalInput")
    out = nc.dram_tensor("out", [T, H], F32, kind="ExternalOutput")
    aux_out = nc.dram_tensor("aux_out", [4, 2], F32, kind="ExternalOutput")

    with TileContext(nc) as tc:
        with (
            tc.tile_pool(name="sb", bufs=1) as sb,
            tc.tile_pool(name="ps", bufs=8, space="PSUM") as ps,
        ):
            # ── constants ──────────────────────────────────────────────
            consts_sb = sb.tile([128, 130], F32, name="consts_sb")
            nc.sync.dma_start(consts_sb[:], consts[:, :])
            ident = consts_sb[:, 0:128]
            ones_col = consts_sb[:, 128:129]
            onesr_sb = sb.tile([1, 128], F32, name="onesr_sb")
            nc.sync.dma_start(onesr_sb[:], ones_row[:, :])
            sel_f = sb.tile([4, 512], F32, name="sel_f")
            nc.sync.dma_start(sel_f[:], sel_mat[:, :])
            sel_r = sb.tile([4, 512], F32R, name="sel_r")
            nc.vector.tensor_copy(sel_r[:], sel_f[:])
            wg_sb = sb.tile([128, NH * E], F32, name="wg_sb")
            nc.sync.dma_start(
                wg_sb[:].rearrange("p (k e) -> p k e", e=E),
                wg[:, :].rearrange("(k p) e -> p k e", p=128),
            )
            bg_sb = sb.tile([1, E], F32, name="bg_sb")
            nc.sync.dma_start(bg_sb[:], bg[:, :])
            b1_sb = sb.tile([128, E * FF // 128], F32, name="b1_sb")  # [128, 256]
            nc.sync.dma_start(
                b1_sb[:].rearrange("p (e c) -> p e c", e=E),
                b1[:, :].rearrange("e (c p) -> p e c", p=128),
            )
            b2_sb = sb.tile([4, H], F32R, name="b2_sb")
            nc.sync.dma_start(b2_sb[:], b2[:, :])

            # ── resident big buffers ───────────────────────────────────
            xt = [sb.tile([128, T], F32R, name=f"xt{h}") for h in range(NH)]
            gateT = sb.tile([4, T], F32R, name="gateT")
            he = [sb.tile([128, T], F32R, name=f"he{f}") for f in range(FM)]

            # ── prologue: transpose X, router, gates ───────────────────
            psum_cnt = ps.tile([4, 1], F32, name="psum_cnt", tag="ps")
            psum_prob = ps.tile([4, 1], F32, name="psum_prob", tag="ps")
            for t8 in range(T8):
                # X tile shares slots with the Y accumulator (same shape).
                xtile = sb.tile([128, H], F32, name=f"y{t8}", tag=f"y{t8}")
                nc.sync.dma_start(xtile[:], hs[t8 * 128 : (t8 + 1) * 128, :])
                psum_l = ps.tile([128, E], F32, name=f"psl{t8}", tag="ps")
                for h16 in range(NH):
                    psum_t = ps.tile([128, 128], F32, name=f"pst{t8}_{h16}", tag="ps")
                    nc.tensor.transpose(
                        psum_t[:], xtile[:, h16 * 128 : (h16 + 1) * 128], ident
                    )
                    nc.vector.tensor_copy(
                        xt[h16][:, t8 * 128 : (t8 + 1) * 128], psum_t[:]
                    )
                    xt32 = sb.tile(
                        [128, 128], F32, name=f"xt32_{t8}_{h16}", tag="xt32", bufs=2
                    )
                    nc.scalar.copy(xt32[:], psum_t[:])
                    nc.tensor.matmul(
                        psum_l[:],
                        xt32[:],
                        wg_sb[:].rearrange("p (k e) -> p k e", e=E)[:, h16, :],
                        start=(h16 == 0),
                        stop=False,
                    )
                nc.tensor.matmul(
                    psum_l[:], onesr_sb[:], bg_sb[:], start=False, stop=True
                )
                lg = sb.tile([128, E], F32, name=f"lg{t8}", tag="lg", bufs=2)
                nc.vector.tensor_copy(lg[:], psum_l[:])
                m1 = sb.tile([128, 1], F32, name=f"m1_{t8}", tag="m1", bufs=2)
                nc.vector.tensor_reduce(m1[:], lg[:], axis=AX.X, op=A.max)
                nm1 = sb.tile([128, 1], F32, name=f"nm1_{t8}", tag="nm1", bufs=2)
                nc.vector.tensor_scalar_mul(nm1[:], m1[:], -1.0)
                is1 = sb.tile([128, E], F32, name=f"is1_{t8}", tag="is1", bufs=2)
                nc.vector.tensor_scalar(is1[:], lg[:], m1[:], None, A.is_equal)
                l2 = sb.tile([128, E], F32, name=f"l2_{t8}", tag="l2", bufs=2)
                nc.vector.scalar_tensor_tensor(
                    l2[:], is1[:], -1e30, lg[:], A.mult, A.add
                )
                m2 = sb.tile([128, 1], F32, name=f"m2_{t8}", tag="m2", bufs=2)
                nc.vector.tensor_reduce(m2[:], l2[:], axis=AX.X, op=A.max)
                is2 = sb.tile([128, E], F32, name=f"is2_{t8}", tag="is2", bufs=2)
                nc.vector.tensor_scalar(is2[:], l2[:], m2[:], None, A.is_equal)
                flags = sb.tile([128, E], F32, name=f"flags{t8}", tag="flags", bufs=2)
                nc.vector.tensor_tensor(flags[:], is1[:], is2[:], A.add)
                eall = sb.tile([128, E], F32, name=f"eall{t8}", tag="eall", bufs=2)
                nc.scalar.activation(eall[:], lg[:], AF.Exp, bias=nm1[:], scale=1.0)
                selp = sb.tile([128, E], F32, name=f"selp{t8}", tag="selp", bufs=2)
                nc.vector.tensor_tensor(selp[:], eall[:], flags[:], A.mult)
                s2 = sb.tile([128, 1], F32, name=f"s2_{t8}", tag="s2", bufs=2)
                nc.vector.tensor_reduce(s2[:], selp[:], axis=AX.X, op=A.add)
                rs2 = sb.tile([128, 1], F32, name=f"rs2_{t8}", tag="rs2", bufs=2)
                nc.vector.reciprocal(rs2[:], s2[:])
                gate = sb.tile([128, E], F32, name=f"gate{t8}", tag="gate", bufs=2)
                nc.vector.tensor_scalar(gate[:], selp[:], rs2[:], None, A.mult)
                z = sb.tile([128, 1], F32, name=f"z{t8}", tag="z", bufs=2)
                nc.vector.tensor_reduce(z[:], eall[:], axis=AX.X, op=A.add)
                rz = sb.tile([128, 1], F32, name=f"rz{t8}", tag="rz", bufs=2)
                nc.vector.reciprocal(rz[:], z[:])
                probs = sb.tile([128, E], F32, name=f"probs{t8}", tag="probs", bufs=2)
                nc.vector.tensor_scalar(probs[:], eall[:], rz[:], None, A.mult)
                nc.tensor.matmul(
                    psum_cnt[:], flags[:], ones_col,
                    start=(t8 == 0), stop=(t8 == T8 - 1),
                )
                nc.tensor.matmul(
                    psum_prob[:], probs[:], ones_col,
                    start=(t8 == 0), stop=(t8 == T8 - 1),
                )
                psum_gt = ps.tile([4, 128], F32, name=f"psgt{t8}", tag="ps")
                nc.tensor.transpose(psum_gt[:], gate[:], ident)
                nc.vector.tensor_copy(gateT[:, t8 * 128 : (t8 + 1) * 128], psum_gt[:])
            aux_sb = sb.tile([4, 2], F32, name="aux_sb")
            nc.vector.tensor_copy(aux_sb[:, 0:1], psum_cnt[:])
            nc.vector.tensor_copy(aux_sb[:, 1:2], psum_prob[:])
            nc.sync.dma_start(aux_out[:, :], aux_sb[:])
            # ── main loop: experts × FF blocks ─────────────────────────
            y = [None] * T8
            for e in range(E):
                # gate_rep[p, t] = gate[t, e] for all p
                grep = sb.tile([128, T], F32R, name=f"grep{e}", tag="grep", bufs=2)
                for n in range(TN):
                    psum_gr = ps.tile([128, 512], F32, name=f"psgr{e}_{n}", tag="ps")
                    nc.tensor.matmul(
                        psum_gr[:],
                        sel_r[:, e * 128 : (e + 1) * 128],
                        gateT[:, n * 512 : (n + 1) * 512],
                        start=True, stop=True,
                    )
                    nc.vector.tensor_copy(
                        grep[:, n * 512 : (n + 1) * 512], psum_gr[:]
                    )
                for b in range(NB):
                    # stage 1: He[fm] = gelu(W1e_chunk.T @ XT + b1) * gate
                    for fm in range(FM):
                        w1b = sb.tile([128, H], F32R, name=f"w1b_{e}_{b}_{fm}",
                                      tag="w1b", bufs=2)
                        c0 = b * FB + fm * 128
                        nc.sync.dma_start(
                            w1b[:].rearrange("p (k c) -> p k c", c=128),
                            w1[e].rearrange("(k p) f -> p k f", p=128)[:, :, c0 : c0 + 128],
                        )
                        for n in range(TN):
                            ph = ps.tile([128, 512], F32, name=f"ph{e}_{b}_{fm}_{n}",
                                         tag="ps")
                            for hk in range(NH):
                                nc.tensor.matmul(
                                    ph[:],
                                    w1b[:, hk * 128 : (hk + 1) * 128],
                                    xt[hk][:, n * 512 : (n + 1) * 512],
                                    start=(hk == 0),
                                    stop=(hk == NH - 1),
                                )
                            gtmp = sb.tile([128, 512], F32, name=f"gt{e}_{b}_{fm}_{n}",
                                           tag="gtmp", bufs=2)
                            bcol = e * (FF // 128) + b * FM + fm
                            nc.scalar.activation(
                                gtmp[:], ph[:], AF.Gelu_apprx_tanh,
                                bias=b1_sb[:, bcol : bcol + 1], scale=1.0,
                            )
                            nc.vector.tensor_tensor(
                                he[fm][:, n * 512 : (n + 1) * 512],
                                gtmp[:],
                                grep[:, n * 512 : (n + 1) * 512],
                                A.mult,
                            )
                    # stage 2: Y[t8][:, hn] (+)= sum_fk He[fk].T_chunk @ W2e_chunk
                    for hn in range(HN):
                        strips = [
                            ps.tile([128, 512], F32, name=f"st{e}_{b}_{hn}_{t8}",
                                    tag="ps")
                            for t8 in range(T8)
                        ]
                        for fk in range(FM):
                            w2t = sb.tile([128, 512], F32R,
                                          name=f"w2t_{e}_{b}_{hn}_{fk}",
                                          tag="w2t", bufs=2)
                            r0 = b * FB + fk * 128
                            nc.sync.dma_start(
                                w2t[:],
                                w2[e, r0 : r0 + 128, hn * 512 : (hn + 1) * 512],
                            )
                            for t8 in range(T8):
                                if fk == 0 and e == 0 and b == 0:
                                    nc.tensor.matmul(
                                        strips[t8],
                                        gateT[:, t8 * 128 : (t8 + 1) * 128],
                                        b2_sb[:, hn * 512 : (hn + 1) * 512],
                                        start=True, stop=False,
                                    )
                                nc.tensor.matmul(
                                    strips[t8],
                                    he[fk][:, t8 * 128 : (t8 + 1) * 128],
                                    w2t[:],
                                    start=(fk == 0 and not (e == 0 and b == 0)),
                                    stop=(fk == FM - 1),
                                )
                        for t8 in range(T8):
                            if e == 0 and b == 0:
                                if y[t8] is None:
                                    y[t8] = sb.tile([128, H], F32, name=f"y{t8}_m",
                                                    tag=f"y{t8}")
                                nc.vector.tensor_copy(
                                    y[t8][:, hn * 512 : (hn + 1) * 512], strips[t8]
                                )
                            else:
                                nc.vector.tensor_add(
                                    y[t8][:, hn * 512 : (hn + 1) * 512],
                                    y[t8][:, hn * 512 : (hn + 1) * 512],
                                    strips[t8],
                                )
            # ── write out ──────────────────────────────────────────────
            for t8 in range(T8):
                nc.sync.dma_start(out[t8 * 128 : (t8 + 1) * 128, :], y[t8][:])
    return nc


_NC_CACHE = None
LAST_RESULTS = None


def kernel(hidden_states, w_gate, b_gate, w1, b1, w2, b2):
    global _NC_CACHE, LAST_RESULTS
    hidden_states = np.ascontiguousarray(np.asarray(hidden_states, dtype=np.float32))
    w_gate = np.ascontiguousarray(np.asarray(w_gate, dtype=np.float32))
    b_gate = np.ascontiguousarray(np.asarray(b_gate, dtype=np.float32)).reshape(1, E)
    w1 = np.ascontiguousarray(np.asarray(w1, dtype=np.float32))
    b1 = np.ascontiguousarray(np.asarray(b1, dtype=np.float32))
    w2 = np.ascontiguousarray(np.asarray(w2, dtype=np.float32))
    b2 = np.ascontiguousarray(np.asarray(b2, dtype=np.float32))

    consts = np.zeros((128, 130), np.float32)
    consts[:, :128] = np.eye(128, dtype=np.float32)
    consts[:, 128] = 1.0
    ones_row = np.ones((1, 128), np.float32)
    sel = np.zeros((4, 512), np.float32)
    for e in range(4):
        sel[e, e * 128 : (e + 1) * 128] = 1.0

    x_flat = hidden_states.reshape(N_TOK, H)

    if _NC_CACHE is None:
        _NC_CACHE = build()
    nc = _NC_CACHE

    in_maps = []
    for c in range(NCORES):
        in_maps.append(
            {
                "hs": np.ascontiguousarray(x_flat[c * T : (c + 1) * T]),
                "wg": w_gate,
                "bg": b_gate,
                "w1": w1,
                "b1": b1,
                "w2": w2,
                "b2": b2,
                "consts": consts,
                "ones_row": ones_row,
                "sel_mat": sel,
            }
        )

    trace = os.environ.get("BASS_MOE_TRACE", "0") == "1"
    res = run_bass_kernel_spmd(
        nc, in_maps, core_ids=list(range(NCORES)), trace=trace
    )
    LAST_RESULTS = res

    out = np.concatenate([res.results[c]["out"] for c in range(NCORES)], axis=0)
    out = out.reshape(B, S, H)

    counts = np.zeros(E, np.float64)
    sump = np.zeros(E, np.float64)
    for c in range(NCORES):
        aux = res.results[c]["aux_out"]
        counts += aux[:, 0].astype(np.float64)
        sump += aux[:, 1].astype(np.float64)
    frac = counts / (N_TOK * K)
    meanp = sump / N_TOK
    aux_loss = np.float32(E * np.sum(frac * meanp))
    return out, aux_loss


# revision 9
# speedup vs baseline: 1.3018x; 1.3018x over previous
"""MoE layer (B=4,S=2048,H=2048,E=4,K=2,FF=8192) on 8 TRN2 NeuronCores.

Strategy: token-parallel. Each core takes 1024 tokens and all 4 experts'
weights, computes the router + dense (masked) expert FFNs locally, and
writes a disjoint slice of the output. No collectives; host just
concatenates slices and reduces the tiny aux-loss partials.

Numerics: router in fp32 (exact top-k), FFN matmuls in float32r
(~1e-4 rel err, full PE rate).
"""

import json
import os

import numpy as np

import concourse.bass as bass
import concourse.mybir as mybir
import concourse.tile as tile_mod
from concourse.bass_utils import run_bass_kernel_spmd
from concourse.tile import TileContext
from concourse.vector_clock import ScopedClock

# ─────────────────────────────────────────────────────────────────────────────
# Workaround for this walrus build's 1-sync-wait-per-instruction limit.
_MAX_WAITS = 1


def _patched_drain_and_barrier(self, tick_clock, wait_clock):
    probe = self.nc.sync.nop(nofuse=True)
    wait_clock.add_sem_waits(probe.ins, ScopedClock({None: tick_clock.global_clock}))
    si = probe.ins.sync_info
    waits = list(si.on_wait) if si is not None else []
    if len(waits) > 1:
        probe.ins.sync_info = mybir.SyncInfo(
            on_wait=[waits[0]], on_update=list(si.on_update)
        )
        for w in waits[1:]:
            n = self.nc.sync.nop(nofuse=True)
            n.ins.sync_info = mybir.SyncInfo(on_wait=[w], on_update=[])
    self.nc.sync.drain()
    self.nc.all_engine_barrier()
    assert self.sems is not None
    popped = self.nc._tile_sem_poison_stack.pop()
    assert popped is self._sem_poison
    self.nc.clear_and_free_semaphores(list(self.sems.allocated().values()))


def _split_waits_in_bir_json(bir_json: bytes) -> bytes:
    m = json.loads(bir_json)
    changed = False
    for f in m.get("functions", []):
        for b in f.get("blocks", []):
            insts = b.get("instructions", [])
            out = []
            for inst in insts:
                si = inst.get("sync_info") or {}
                waits = si.get("on_wait") or []
                if len(waits) > _MAX_WAITS:
                    changed = True
                    extra, keep = waits[:-_MAX_WAITS], waits[-_MAX_WAITS:]
                    for j, w in enumerate(extra):
                        out.append(
                            {
                                "debug": inst.get("debug"),
                                "engine": inst["engine"],
                                "ins": [],
                                "name": f"{inst['name']}-wsplit{j}",
                                "opcode": "NoOp",
                                "outs": [],
                                "sync_info": {"on_update": [], "on_wait": [w]},
                            }
                        )
                    si = dict(si)
                    si["on_wait"] = keep
                    inst = dict(inst)
                    inst["sync_info"] = si
                out.append(inst)
            b["instructions"] = out
    if not changed:
        return bir_json
    return json.dumps(m).encode()


def _install_fixes():
    tile_mod.TileContext._drain_and_barrier = _patched_drain_and_barrier
    import concourse.bass_utils as bu
    import concourse.bass2jax as b2j

    if getattr(bu, "_wsplit_installed", False):
        return
    orig = bu.compile_bir_kernel

    def wrapped(bir_json, tmpdir, neff_name="file.neff"):
        return orig(_split_waits_in_bir_json(bytes(bir_json)), tmpdir, neff_name)

    bu.compile_bir_kernel = wrapped
    bu._wsplit_installed = True
    if getattr(b2j, "compile_bir_kernel", None) is orig:
        b2j.compile_bir_kernel = wrapped


_install_fixes()

# ─────────────────────────────────────────────────────────────────────────────

F32 = mybir.dt.float32
F32R = mybir.dt.float32r
A = mybir.AluOpType
AX = mybir.AxisListType
AF = mybir.ActivationFunctionType

B, S, H = 4, 2048, 2048
E, K = 4, 2
FF = 4 * H
NCORES = 8
N_TOK = B * S
T = N_TOK // NCORES  # 1024 tokens per core
T8 = T // 128  # 8
NH = H // 128  # 16
FB = 512  # FF block
NB = FF // FB  # 8
FM = FB // 128  # 8
HN = H // 512  # 4
TN = T // 512  # 2


def build():
    nc = bass.Bass()
    hs = nc.dram_tensor("hs", [T, H], F32, kind="ExternalInput")
    wg = nc.dram_tensor("wg", [H, E], F32, kind="ExternalInput")
    bg = nc.dram_tensor("bg", [1, E], F32, kind="ExternalInput")
    w1 = nc.dram_tensor("w1", [E, H, FF], F32R, kind="ExternalInput")
    b1 = nc.dram_tensor("b1", [E, FF], F32, kind="ExternalInput")
    w2 = nc.dram_tensor("w2", [E, FF, H], F32R, kind="ExternalInput")
    b2 = nc.dram_tensor("b2", [E, H], F32R, kind="ExternalInput")
    consts = nc.dram_tensor("consts", [128, 130], F32, kind="ExternalInput")
    ones_row = nc.dram_tensor("ones_row", [1, 128], F32, kind="ExternalInput")
    sel_mat = nc.dram_tensor("sel_mat", [4, 512], F32, kind="ExternalInput")
    out = nc.dram_tensor("out", [T, H], F32, kind="ExternalOutput")
    aux_out = nc.dram_tensor("aux_out", [4, 2], F32, kind="ExternalOutput")

    with TileContext(nc) as tc:
        with (
            tc.tile_pool(name="sb", bufs=1) as sb,
            tc.tile_pool(name="ps", bufs=8, space="PSUM") as ps,
        ):
            # ── constants ──────────────────────────────────────────────
            consts_sb = sb.tile([128, 130], F32, name="consts_sb")
            nc.sync.dma_start(consts_sb[:], consts[:, :])
            ident = consts_sb[:, 0:128]
            ones_col = consts_sb[:, 128:129]
            onesr_sb = sb.tile([1, 128], F32, name="onesr_sb")
            nc.sync.dma_start(onesr_sb[:], ones_row[:, :])
            sel_f = sb.tile([4, 512], F32, name="sel_f")
            nc.sync.dma_start(sel_f[:], sel_mat[:, :])
            sel_r = sb.tile([4, 512], F32R, name="sel_r")
            nc.vector.tensor_copy(sel_r[:], sel_f[:])
            wg_sb = sb.tile([128, NH * E], F32, name="wg_sb")
            nc.sync.dma_start(
                wg_sb[:].rearrange("p (k e) -> p k e", e=E),
                wg[:, :].rearrange("(k p) e -> p k e", p=128),
            )
            bg_sb = sb.tile([1, E], F32, name="bg_sb")
            nc.sync.dma_start(bg_sb[:], bg[:, :])
            b1_sb = sb.tile([128, E * FF // 128], F32, name="b1_sb")  # [128, 256]
            nc.sync.dma_start(
                b1_sb[:].rearrange("p (e c) -> p e c", e=E),
                b1[:, :].rearrange("e (c p) -> p e c", p=128),
            )
            b2_sb = sb.tile([4, H], F32R, name="b2_sb")
            nc.sync.dma_start(b2_sb[:], b2[:, :])

            # ── resident big buffers ───────────────────────────────────
            xt = [sb.tile([128, T], F32R, name=f"xt{h}") for h in range(NH)]
            gateT = sb.tile([4, T], F32R, name="gateT")
            he = [sb.tile([128, T], F32R, name=f"he{f}") for f in range(FM)]

            # ── prologue: transpose X, router, gates ───────────────────
            psum_cnt = ps.tile([4, 1], F32, name="psum_cnt", tag="ps")
            psum_prob = ps.tile([4, 1], F32, name="psum_prob", tag="ps")
            for t8 in range(T8):
                # X half-tiles share slots with the Yt accumulator (same shape).
                xhalf = []
                for hh in range(2):
                    xh = sb.tile([128, H // 2], F32, name=f"yt{2*t8+hh}",
                                 tag=f"yt{2*t8+hh}")
                    nc.sync.dma_start(
                        xh[:],
                        hs[t8 * 128 : (t8 + 1) * 128,
                           hh * (H // 2) : (hh + 1) * (H // 2)],
                    )
                    xhalf.append(xh)
                psum_l = ps.tile([128, E], F32, name=f"psl{t8}", tag="ps")
                for h16 in range(NH):
                    psum_t = ps.tile([128, 128], F32, name=f"pst{t8}_{h16}", tag="ps")
                    nc.tensor.transpose(
                        psum_t[:],
                        xhalf[h16 // 8][:, (h16 % 8) * 128 : (h16 % 8 + 1) * 128],
                        ident,
                    )
                    nc.vector.tensor_copy(
                        xt[h16][:, t8 * 128 : (t8 + 1) * 128], psum_t[:]
                    )
                    xt32 = sb.tile(
                        [128, 128], F32, name=f"xt32_{t8}_{h16}", tag="xt32", bufs=2
                    )
                    nc.scalar.copy(xt32[:], psum_t[:])
                    nc.tensor.matmul(
                        psum_l[:],
                        xt32[:],
                        wg_sb[:].rearrange("p (k e) -> p k e", e=E)[:, h16, :],
                        start=(h16 == 0),
                        stop=False,
                    )
                nc.tensor.matmul(
                    psum_l[:], onesr_sb[:], bg_sb[:], start=False, stop=True
                )
                lg = sb.tile([128, E], F32, name=f"lg{t8}", tag="lg", bufs=1)
                nc.vector.tensor_copy(lg[:], psum_l[:])
                m1 = sb.tile([128, 1], F32, name=f"m1_{t8}", tag="m1", bufs=1)
                nc.vector.tensor_reduce(m1[:], lg[:], axis=AX.X, op=A.max)
                nm1 = sb.tile([128, 1], F32, name=f"nm1_{t8}", tag="nm1", bufs=1)
                nc.vector.tensor_scalar_mul(nm1[:], m1[:], -1.0)
                is1 = sb.tile([128, E], F32, name=f"is1_{t8}", tag="is1", bufs=1)
                nc.vector.tensor_scalar(is1[:], lg[:], m1[:], None, A.is_equal)
                l2 = sb.tile([128, E], F32, name=f"l2_{t8}", tag="l2", bufs=1)
                nc.vector.scalar_tensor_tensor(
                    l2[:], is1[:], -1e30, lg[:], A.mult, A.add
                )
                m2 = sb.tile([128, 1], F32, name=f"m2_{t8}", tag="m2", bufs=1)
                nc.vector.tensor_reduce(m2[:], l2[:], axis=AX.X, op=A.max)
                is2 = sb.tile([128, E], F32, name=f"is2_{t8}", tag="is2", bufs=1)
                nc.vector.tensor_scalar(is2[:], l2[:], m2[:], None, A.is_equal)
                flags = sb.tile([128, E], F32, name=f"flags{t8}", tag="flags", bufs=1)
                nc.vector.tensor_tensor(flags[:], is1[:], is2[:], A.add)
                eall = sb.tile([128, E], F32, name=f"eall{t8}", tag="eall", bufs=1)
                nc.scalar.activation(eall[:], lg[:], AF.Exp, bias=nm1[:], scale=1.0)
                selp = sb.tile([128, E], F32, name=f"selp{t8}", tag="selp", bufs=1)
                nc.vector.tensor_tensor(selp[:], eall[:], flags[:], A.mult)
                s2 = sb.tile([128, 1], F32, name=f"s2_{t8}", tag="s2", bufs=1)
                nc.vector.tensor_reduce(s2[:], selp[:], axis=AX.X, op=A.add)
                rs2 = sb.tile([128, 1], F32, name=f"rs2_{t8}", tag="rs2", bufs=1)
                nc.vector.reciprocal(rs2[:], s2[:])
                gate = sb.tile([128, E], F32, name=f"gate{t8}", tag="gate", bufs=1)
                nc.vector.tensor_scalar(gate[:], selp[:], rs2[:], None, A.mult)
                z = sb.tile([128, 1], F32, name=f"z{t8}", tag="z", bufs=1)
                nc.vector.tensor_reduce(z[:], eall[:], axis=AX.X, op=A.add)
                rz = sb.tile([128, 1], F32, name=f"rz{t8}", tag="rz", bufs=1)
                nc.vector.reciprocal(rz[:], z[:])
                probs = sb.tile([128, E], F32, name=f"probs{t8}", tag="probs", bufs=1)
                nc.vector.tensor_scalar(probs[:], eall[:], rz[:], None, A.mult)
                nc.tensor.matmul(
                    psum_cnt[:], flags[:], ones_col,
                    start=(t8 == 0), stop=(t8 == T8 - 1),
                )
                nc.tensor.matmul(
                    psum_prob[:], probs[:], ones_col,
                    start=(t8 == 0), stop=(t8 == T8 - 1),
                )
                psum_gt = ps.tile([4, 128], F32, name=f"psgt{t8}", tag="ps")
                nc.tensor.transpose(psum_gt[:], gate[:], ident)
                nc.vector.tensor_copy(gateT[:, t8 * 128 : (t8 + 1) * 128], psum_gt[:])
            aux_sb = sb.tile([4, 2], F32, name="aux_sb")
            nc.vector.tensor_copy(aux_sb[:, 0:1], psum_cnt[:])
            nc.vector.tensor_copy(aux_sb[:, 1:2], psum_prob[:])
            nc.sync.dma_start(aux_out[:, :], aux_sb[:])
            # ── main loop: experts × FF blocks ─────────────────────────
            yt = [None] * NH
            for e in range(E):
                # gate_rep[p, t] = gate[t, e] for all p
                grep = sb.tile([128, T], F32R, name=f"grep{e}", tag="grep", bufs=2)
                for n in range(TN):
                    psum_gr = ps.tile([128, 512], F32, name=f"psgr{e}_{n}", tag="ps")
                    nc.tensor.matmul(
                        psum_gr[:],
                        sel_r[:, e * 128 : (e + 1) * 128],
                        gateT[:, n * 512 : (n + 1) * 512],
                        start=True, stop=True,
                    )
                    nc.vector.tensor_copy(
                        grep[:, n * 512 : (n + 1) * 512], psum_gr[:]
                    )
                for b in range(NB):
                    # stage 1: He[fm] = gelu(W1e_chunk.T @ XT + b1) * gate
                    for fm in range(FM):
                        w1b = sb.tile([128, H], F32R, name=f"w1b_{e}_{b}_{fm}",
                                      tag="w1b", bufs=2)
                        c0 = b * FB + fm * 128
                        nc.sync.dma_start(
                            w1b[:].rearrange("p (k c) -> p k c", c=128),
                            w1[e, :, :].rearrange("(k p) f -> p k f", p=128)[:, :, c0 : c0 + 128],
                        )
                        phn = [
                            ps.tile([128, 512], F32, name=f"ph{e}_{b}_{fm}_{n}",
                                    tag="ps")
                            for n in range(TN)
                        ]
                        for hk in range(NH):
                            for n in range(TN):
                                nc.tensor.matmul(
                                    phn[n][:],
                                    w1b[:, hk * 128 : (hk + 1) * 128],
                                    xt[hk][:, n * 512 : (n + 1) * 512],
                                    start=(hk == 0),
                                    stop=(hk == NH - 1),
                                )
                        for n in range(TN):
                            gtmp = sb.tile([128, 512], F32, name=f"gt{e}_{b}_{fm}_{n}",
                                           tag="gtmp", bufs=2)
                            bcol = e * (FF // 128) + b * FM + fm
                            nc.scalar.activation(
                                gtmp[:], phn[n][:], AF.Gelu_apprx_tanh,
                                bias=b1_sb[:, bcol : bcol + 1], scale=1.0,
                            )
                            nc.vector.tensor_tensor(
                                he[fm][:, n * 512 : (n + 1) * 512],
                                gtmp[:],
                                grep[:, n * 512 : (n + 1) * 512],
                                A.mult,
                            )
                    # stage 2: Yt[hm] (+)= sum_fk W2e_chunk.T @ He[fk]
                    for hm in range(NH):
                        w2c = sb.tile([128, FM * 128], F32R,
                                      name=f"w2c_{e}_{b}_{hm}", tag="w2c", bufs=3)
                        nc.sync.dma_start(
                            w2c[:].rearrange("p (q c) -> p q c", c=128),
                            w2[e, :, :].rearrange("(q p) h -> p q h", p=128)[
                                :, b * FM : (b + 1) * FM,
                                hm * 128 : (hm + 1) * 128,
                            ],
                        )
                        ytn = [
                            ps.tile([128, 512], F32, name=f"yp{e}_{b}_{hm}_{n}",
                                    tag="ps")
                            for n in range(TN)
                        ]
                        first_eb = e == 0 and b == 0
                        if first_eb:
                            for n in range(TN):
                                nc.tensor.matmul(
                                    ytn[n][:],
                                    b2_sb[:, hm * 128 : (hm + 1) * 128],
                                    gateT[:, n * 512 : (n + 1) * 512],
                                    start=True, stop=False,
                                )
                        for fk in range(FM):
                            for n in range(TN):
                                nc.tensor.matmul(
                                    ytn[n][:],
                                    w2c[:, fk * 128 : (fk + 1) * 128],
                                    he[fk][:, n * 512 : (n + 1) * 512],
                                    start=(fk == 0 and not first_eb),
                                    stop=(fk == FM - 1),
                                )
                        for n in range(TN):
                            if first_eb:
                                if yt[hm] is None:
                                    yt[hm] = sb.tile([128, T], F32,
                                                     name=f"yt{hm}_m", tag=f"yt{hm}")
                                nc.vector.tensor_copy(
                                    yt[hm][:, n * 512 : (n + 1) * 512], ytn[n][:]
                                )
                            else:
                                nc.vector.tensor_add(
                                    yt[hm][:, n * 512 : (n + 1) * 512],
                                    yt[hm][:, n * 512 : (n + 1) * 512],
                                    ytn[n][:],
                                )
            # ── write out: transpose Yt back to [tok, H] ───────────────
            for t8 in range(T8):
                yout = sb.tile([128, H], F32, name=f"yout{t8}", tag="yout", bufs=1)
                for hm in range(NH):
                    psum_o = ps.tile([128, 128], F32, name=f"pso{t8}_{hm}", tag="ps")
                    nc.tensor.transpose(
                        psum_o[:], yt[hm][:, t8 * 128 : (t8 + 1) * 128], ident
                    )
                    nc.vector.tensor_copy(yout[:, hm * 128 : (hm + 1) * 128], psum_o[:])
                nc.sync.dma_start(out[t8 * 128 : (t8 + 1) * 128, :], yout[:])
    return nc


_NC_CACHE = None
LAST_RESULTS = None


def kernel(hidden_states, w_gate, b_gate, w1, b1, w2, b2):
    global _NC_CACHE, LAST_RESULTS
    hidden_states = np.ascontiguousarray(np.asarray(hidden_states, dtype=np.float32))
    w_gate = np.ascontiguousarray(np.asarray(w_gate, dtype=np.float32))
    b_gate = np.ascontiguousarray(np.asarray(b_gate, dtype=np.float32)).reshape(1, E)
    w1 = np.ascontiguousarray(np.asarray(w1, dtype=np.float32))
    b1 = np.ascontiguousarray(np.asarray(b1, dtype=np.float32))
    w2 = np.ascontiguousarray(np.asarray(w2, dtype=np.float32))
    b2 = np.ascontiguousarray(np.asarray(b2, dtype=np.float32))

    consts = np.zeros((128, 130), np.float32)
    consts[:, :128] = np.eye(128, dtype=np.float32)
    consts[:, 128] = 1.0
    ones_row = np.ones((1, 128), np.float32)
    sel = np.zeros((4, 512), np.float32)
    for e in range(4):
        sel[e, e * 128 : (e + 1) * 128] = 1.0

    x_flat = hidden_states.reshape(N_TOK, H)

    if _NC_CACHE is None:
        _NC_CACHE = build()
    nc = _NC_CACHE

    in_maps = []
    for c in range(NCORES):
        in_maps.append(
            {
                "hs": np.ascontiguousarray(x_flat[c * T : (c + 1) * T]),
                "wg": w_gate,
                "bg": b_gate,
                "w1": w1,
                "b1": b1,
                "w2": w2,
                "b2": b2,
                "consts": consts,
                "ones_row": ones_row,
                "sel_mat": sel,
            }
        )

    trace = os.environ.get("BASS_MOE_TRACE", "0") == "1"
    res = run_bass_kernel_spmd(
        nc, in_maps, core_ids=list(range(NCORES)), trace=trace
    )
    LAST_RESULTS = res

    out = np.concatenate([res.results[c]["out"] for c in range(NCORES)], axis=0)
    out = out.reshape(B, S, H)

    counts = np.zeros(E, np.float64)
    sump = np.zeros(E, np.float64)
    for c in range(NCORES):
        aux = res.results[c]["aux_out"]
        counts += aux[:, 0].astype(np.float64)
        sump += aux[:, 1].astype(np.float64)
    frac = counts / (N_TOK * K)
    meanp = sump / N_TOK
    aux_loss = np.float32(E * np.sum(frac * meanp))
    return out, aux_loss
